# revision 43
# baseline (speedup 1.0000x reference)
"""Trainium2 Bass kernel for nn_CausalAttention_50629074485540.

Causal MHA (B=2, T=2048, D=1024, H=16, hd=64) with ALiBi, tensor-parallel
over heads on 8 cores (2 heads/core): Wq/Wk/Wv column-sharded, Wo
row-sharded, x replicated, all matmuls fp16, per-core partials summed
host-side.  Scores are computed transposed with the ALiBi bias + row
stabilizer folded into 4 extra contraction rows (fp16 2-splits), so the
PE accumulates q.k + 8*slope*(j-i) directly; exp(s/8) comes straight out
of PSUM on ACT; V carries an appended ones column so attn^T = V~^T @ P
accumulates the softmax denominator for free.

Optimizations vs the first working version (cost-model 175955 -> 144720 ns;
HW repetition-slope approx 130-175 us, relay-noise limited):
  - diagonal-tile column restriction: scores matmul / exp / attnV only
    touch columns >= 128*a of the 512-wide i-chunk (the rest is fully
    masked); causal clamp shrinks to one [128, 2, 128] inf-safe min
    against a triangle mask (both heads in one DVE op).
  - software-pipelined PE stream: projection work for the next chunk,
    the next-next chunk's Q/K projection (a no-deadline "soft" queue),
    and the previous block's output projection sit in background queues
    drained 2-3 instructions per attention tile between the scores and
    attnV matmuls (which must wait an exp latency anyway; out-proj
    items alternated with proj items), so the in-order PE stream always
    has independent work while ACT exp / DVE drains catch up.
    Flush-critical projections complete at block end (the next block
    needs qt/kt/vt); out-proj and soft work may spill across blocks.
    The final block's out-proj drains via ACT (idle at the tail).
  - DMA: xT host-prepped as [TI, P, KC, 512] and yT written as
    [TI, P, ECH, 512] so every DMA moves 8KB contiguous per partition;
    the xT stream owns the SP queue while wk/wv/wo issue from the ACT
    queue and the ext rows from the gpsimd queue, so small weight
    transfers slot in between the big xT chunks instead of serializing
    the issue path (-9.5us cold-start).
Engine budget (cost model, 1 rep): PE 115us (79%), DVE 97us, ACT 79us,
SP/DMA 69us, Pool 7us.  TRN2 notes: Pool/GPSIMD cannot touch PSUM, and
matmul output must be fp32 psum (fp16 psum is transpose-only).
"""

import math
import sys

import numpy as np

for _p in ("/opt/trn_rl_repo", "/root/.axon_site/_ro/trn_rl_repo"):
    if _p not in sys.path:
        sys.path.append(_p)

import concourse.mybir as mybir
import concourse.tile as tile
from concourse import bacc, bass_utils
from concourse.bass import ts, ds
from concourse.masks import make_identity

F16 = mybir.dt.float16
F32 = mybir.dt.float32

B = 2
T = 2048
D = 1024
HD = 64
H = 16
N_CORES = 8
P = 128
KC = D // P          # 8 contraction chunks for projections
ECH = D // P         # 8 output-projection column chunks
CEXT = 68            # extended score contraction: 64 qk dims + 4 bias rows


def get_slopes(n):
    def pow2(n):
        start = 2 ** (-(2 ** (-(math.log2(n) - 3))))
        return [start * start**i for i in range(n)]
    if math.log2(n).is_integer():
        return pow2(n)
    c = 2 ** math.floor(math.log2(n))
    return pow2(c) + get_slopes(2 * c)[0::2][: n - c]


def build_nc(reps=1):
    """Build the per-core Bass program (identical program on all cores)."""
    BT = B * T
    TJ = T // P           # j-tiles per batch
    NCI = T // 512        # 512-wide i-chunks per batch
    TI = BT // 512        # 512-wide chunks over the full B*T axis

    nc = bacc.Bacc("TRN2", target_bir_lowering=False, debug=False,
                   enable_asserts=True, num_devices=N_CORES)

    xT = nc.dram_tensor("xT", [TI, P, KC, 512], F16, kind="ExternalInput").ap()
    wq = nc.dram_tensor("wq", [D, P], F16, kind="ExternalInput").ap()
    wk = nc.dram_tensor("wk", [D, P], F16, kind="ExternalInput").ap()
    wv = nc.dram_tensor("wv", [D, P], F16, kind="ExternalInput").ap()
    wo = nc.dram_tensor("wo", [P, D], F16, kind="ExternalInput").ap()
    qext = nc.dram_tensor("qext", [2, 4, BT], F16, kind="ExternalInput").ap()
    kext = nc.dram_tensor("kext", [2, 4, BT], F16, kind="ExternalInput").ap()
    yT = nc.dram_tensor("yT", [TI, P, ECH, 512], F16, kind="ExternalOutput").ap()

    wq_t = wq.rearrange("(kc p) m -> p kc m", p=P)
    wk_t = wk.rearrange("(kc p) m -> p kc m", p=P)
    wv_t = wv.rearrange("(kc p) m -> p kc m", p=P)

    def xt_view(ti):
        return xT[ti].rearrange("p kc c -> p (kc c)") \
                     .rearrange("p (kc c) -> p kc c", kc=KC)

    with tile.TileContext(nc) as tc:
        with tc.tile_pool(name="big", bufs=1) as big, \
             tc.tile_pool(name="ptiles", bufs=8) as ptiles, \
             tc.tile_pool(name="mtiles", bufs=4) as mtiles, \
             tc.tile_pool(name="ytiles", bufs=3) as ytiles, \
             tc.tile_pool(name="ntiles", bufs=4) as ntiles, \
             tc.tile_pool(name="vstage", bufs=2) as vstage_pool, \
             tc.tile_pool(name="pp", bufs=2, space="PSUM") as pp, \
             tc.tile_pool(name="ps_s", bufs=2, space="PSUM") as ps_s, \
             tc.tile_pool(name="ps_att", bufs=2, space="PSUM") as ps_att:

            # ---- persistent SBUF buffers ----
            xt_sb = big.tile([P, KC, BT], F16, tag="xt")
            wq_sb = big.tile([P, KC, P], F16, tag="wq")
            wk_sb = big.tile([P, KC, P], F16, tag="wk")
            wv_sb = big.tile([P, KC, P], F16, tag="wv")
            wo_sb = big.tile([P, D], F16, tag="wo")
            ident = big.tile([P, P], F16, tag="ident")
            # per-local-head Q~ / K~ [128, BT]: rows 0-63 head dims, 64-67 ext
            qt = [big.tile([P, BT], F16, tag=f"qt{h}", name=f"qt{h}")
                  for h in range(2)]
            kt = [big.tile([P, BT], F16, tag=f"kt{h}", name=f"kt{h}")
                  for h in range(2)]
            # V~ tiles: [j 128, b, tj, h, 65]; col 64 of each head = ones
            vt = big.tile([P, B, TJ, 2, HD + 1], F16, tag="vt")

            make_identity(nc, ident[:])
            nc.gpsimd.memset(vt[:, :, :, :, HD], 1.0)
            # lower-triangle clamp mask for the one diagonal-crossing
            # [128,128] block of each diagonal tile (inf-safe min)
            tri = big.tile([P, 2, P], F16, tag="tri")
            nc.gpsimd.memset(tri[:], 60000.0)
            for h in range(2):
                nc.gpsimd.affine_select(
                    out=tri[:, h, :], in_=tri[:, h, :],
                    compare_op=mybir.AluOpType.is_ge, fill=0.0,
                    base=0, pattern=[[1, P]], channel_multiplier=-1)

            for _rep in range(reps):
                # critical-path-ordered input DMA: each weight lands just
                # before the projection that reads it
                # xT stream on the SP queue; small weights on the ACT
                # queue and the ext rows on the DVE queue so their issue
                # paths don't serialize behind the big xT transfers
                nc.sync.dma_start(wq_sb[:], wq_t)
                nc.scalar.dma_start(wv_sb[:], wv_t)
                nc.scalar.dma_start(wk_sb[:], wk_t)
                nc.sync.dma_start(xt_sb[:, 0:2, ts(0, 512)], xt_view(0)[:, 0:2])
                nc.sync.dma_start(xt_sb[:, 2:, ts(0, 512)], xt_view(0)[:, 2:])
                for h in range(2):
                    nc.gpsimd.dma_start(qt[h][64:68, :], qext[h])
                    nc.gpsimd.dma_start(kt[h][64:68, :], kext[h])
                nc.scalar.dma_start(wo_sb[:], wo[:])
                for ti in range(1, TI):
                    nc.sync.dma_start(xt_sb[:, :, ts(ti, 512)], xt_view(ti))

                # ---- background PE work queues (one PE inst per item) ----
                bg_proj = []
                bg_out = []
                bg_soft = []   # next-next chunk QK work: no flush deadline

                def push_proj(ti, parts="all", q=None):
                    # Q and K projections: 8 accumulating matmuls each, then
                    # two DVE copies into the per-head qt/kt tiles
                    if q is None:
                        q = bg_proj
                    for w_sb, dst in (((wq_sb, qt), (wk_sb, kt))
                                      if parts in ("all", "qk") else ()):
                        ps = [None]

                        def mk(kc, w_sb=w_sb, dst=dst, ps=ps):
                            if kc == 0:
                                ps[0] = pp.tile([P, 512], F32, tag="proj",
                                                name="ps")
                            nc.tensor.matmul(ps[0][:], w_sb[:, kc, :],
                                             xt_sb[:, kc, ts(ti, 512)],
                                             start=(kc == 0),
                                             stop=(kc == KC - 1))
                            if kc == KC - 1:
                                nc.vector.tensor_copy(
                                    dst[0][0:64, ts(ti, 512)], ps[0][0:64, :])
                                nc.vector.tensor_copy(
                                    dst[1][0:64, ts(ti, 512)], ps[0][64:128, :])
                        for kc in range(KC):
                            q.append(lambda kc=kc, mk=mk: mk(kc))
                    if parts == "qk":
                        return
                    # V^T chunk, staged to SBUF fp16
                    ps = [None]
                    vst = [None]

                    def mkv(kc, ps=ps, vst=vst):
                        if kc == 0:
                            ps[0] = pp.tile([P, 512], F32, tag="proj",
                                            name="ps")
                        nc.tensor.matmul(ps[0][:], wv_sb[:, kc, :],
                                         xt_sb[:, kc, ts(ti, 512)],
                                         start=(kc == 0), stop=(kc == KC - 1))
                        if kc == KC - 1:
                            vst[0] = vstage_pool.tile([P, 512], F16, tag="vst",
                                                      name="vst")
                            nc.vector.tensor_copy(vst[0][:], ps[0][:])
                    for kc in range(KC):
                        bg_proj.append(lambda kc=kc, mkv=mkv: mkv(kc))
                    # PE transpose into V-natural tiles, Pool copy out

                    def mkt(tt, vst=vst):
                        gt = ti * 4 + tt            # global 128-tile over B*T
                        b, tj = divmod(gt, TJ)
                        ps_tr = pp.tile([P, P], F16, tag="proj", name="ps_tr")
                        nc.tensor.transpose(ps_tr[:], vst[0][:, ts(tt, P)],
                                            ident[:])
                        nc.vector.tensor_copy(
                            vt[:, b, tj, :, 0:HD],
                            ps_tr[:].rearrange("p (h c) -> p h c", h=2))
                    for tt in range(4):
                        bg_proj.append(lambda tt=tt, mkt=mkt: mkt(tt))

                def push_outproj(b, ci, merged, tail=False):
                    ysb = ytiles.tile([P, ECH, 512], F16, tag="ysb",
                                      name="ysb")

                    def mko(ec):
                        y_ps = pp.tile([P, 512], F32, tag="proj", name="y_ps")
                        nc.tensor.matmul(y_ps[:], wo_sb[:, ts(ec, P)],
                                         merged[:], start=True, stop=True)
                        # Pool cannot read PSUM on TRN2; DVE drains, except
                        # the final block where ACT is idle (exps all done)
                        if tail:
                            nc.scalar.copy(ysb[:, ec, :], y_ps[:])
                        else:
                            nc.vector.tensor_copy(ysb[:, ec, :], y_ps[:])
                        if ec == ECH - 1:
                            blk = b * NCI + ci
                            yT_v = yT[blk].rearrange("p e c -> p (e c)") \
                                          .rearrange("p (e c) -> p e c", e=ECH)
                            if blk >= B * NCI - 2:
                                for eh in range(4):
                                    nc.sync.dma_start(yT_v[:, ts(eh, 2), :],
                                                      ysb[:, ts(eh, 2), :])
                            else:
                                nc.sync.dma_start(yT_v, ysb[:])
                    for ec in range(ECH):
                        bg_out.append(lambda ec=ec, mko=mko: mko(ec))

                def drain(n):
                    take_out = True
                    for _ in range(min(n, len(bg_proj) + len(bg_out)
                                       + len(bg_soft))):
                        if take_out and bg_out:
                            bg_out.pop(0)()
                        elif bg_proj:
                            bg_proj.pop(0)()
                        elif bg_out:
                            bg_out.pop(0)()
                        elif bg_soft:
                            bg_soft.pop(0)()
                        take_out = not take_out

                def emit_attn(b, ci):
                    # two heads share one exp per j-tile; diagonal tiles only
                    # compute columns >= 128*a
                    i0 = b * T + ci * 512
                    ntj = 4 * ci + 4
                    merged = mtiles.tile([P, 512], F16, tag="merged",
                                         name="merged")
                    att = [ps_att.tile([HD + 1, 512], F32, tag="att",
                                       name="att") for _ in range(2)]
                    for tj in range(ntj):
                        a = tj - 4 * ci             # >=0 on diagonal tiles
                        c0 = max(0, a) * P          # first live column
                        cw = 512 - c0               # live width
                        s2 = ps_s.tile([P, 2, 512], F32, tag="s", name="s2")
                        for h in range(2):
                            nc.tensor.matmul(s2[:, h, c0:512],
                                             kt[h][0:CEXT,
                                                   ds(b * T + tj * P, P)],
                                             qt[h][0:CEXT, ds(i0 + c0, cw)],
                                             start=True, stop=True)
                        pt2 = ptiles.tile([P, 2, 512], F16, tag="pt",
                                          name="pt2")
                        nc.scalar.activation(pt2[:, :, c0:512], s2[:, :, c0:512],
                                             mybir.ActivationFunctionType.Exp,
                                             bias=0.0, scale=0.125)
                        if a >= 0:
                            nc.vector.tensor_tensor(
                                pt2[:, :, c0:c0 + P], pt2[:, :, c0:c0 + P],
                                tri[:], mybir.AluOpType.min)
                        for h in range(2):
                            nc.tensor.matmul(att[h][:, c0:512],
                                             vt[:, b, tj, h, :],
                                             pt2[:, h, c0:512],
                                             start=(tj == 0),
                                             stop=(tj == ntj - 1))
                        # keep PE fed with independent background work
                        left = ntj - 1 - tj
                        if left > 0:
                            n = len(bg_proj) + len(bg_out)
                            drain(min(3, max(2, -(-n // left))) if n else 0)
                    # flush proj work: the next block needs qt/kt/vt complete
                    while bg_proj:
                        drain(1)
                    for h in range(2):
                        recip = ntiles.tile([1, 512], F32, tag="recip",
                                            name="recip")
                        nc.vector.reciprocal(recip[:], att[h][HD:HD + 1, :])
                        rb = ntiles.tile([HD, 512], F32, tag="rb", name="rb")
                        nc.gpsimd.partition_broadcast(rb[:], recip[:])
                        nc.vector.tensor_mul(out=merged[ts(h, HD), :],
                                             in0=att[h][0:HD, :], in1=rb[:])
                    return merged

                push_proj(0)
                while bg_proj:
                    drain(1)        # first projection has nothing to hide in
                push_proj(1, parts="qk", q=bg_soft)
                blk = 0
                for b in range(B):
                    for ci in range(NCI):
                        if blk + 1 < TI:
                            # soft QK items of blk+1 may still be queued;
                            # promote them to the flush-critical queue
                            bg_proj.extend(bg_soft)
                            del bg_soft[:]
                            push_proj(blk + 1, parts="v")
                        if blk + 2 < TI:
                            push_proj(blk + 2, parts="qk", q=bg_soft)
                        merged = emit_attn(b, ci)
                        push_outproj(b, ci, merged,
                                     tail=(blk == B * NCI - 1))
                        blk += 1
                drain(len(bg_proj) + len(bg_out) + len(bg_soft))

    nc.compile()
    return nc


def make_core_inputs(x, Wq, Wk, Wv, Wo, core):
    """Build the fp16 input dict for one core. x: [B, T, D] fp32."""
    BT = B * T
    TI = BT // 512
    xT = x.reshape(BT, D).T.astype(np.float16)               # [D, BT]
    xT = np.ascontiguousarray(
        xT.reshape(KC, P, TI, 512).transpose(2, 1, 0, 3))    # [TI,P,KC,512]
    slopes = np.array(get_slopes(H), dtype=np.float64)
    sl = slice(P * core, P * (core + 1))
    ins = {
        "xT": xT,
        "wq": np.ascontiguousarray(Wq[:, sl]).astype(np.float16),
        "wk": np.ascontiguousarray(Wk[:, sl]).astype(np.float16),
        "wv": np.ascontiguousarray(Wv[:, sl]).astype(np.float16),
        "wo": np.ascontiguousarray(Wo[sl, :]).astype(np.float16),
    }
    pos = np.arange(T, dtype=np.float64)
    qe = np.zeros((2, 4, BT), np.float16)
    ke = np.zeros((2, 4, BT), np.float16)
    for h in range(2):
        g = 2 * core + h
        v = 8.0 * slopes[g] * (pos - 1024.0)       # j-side bias, fp16 2-split
        w = 8.0 * slopes[g] * (1024.0 - pos)       # i-side bias, fp16 2-split
        v1 = v.astype(np.float16)
        v2 = (v - v1.astype(np.float64)).astype(np.float16)
        w1 = w.astype(np.float16)
        w2 = (w - w1.astype(np.float64)).astype(np.float16)
        one = np.ones(T, np.float16)
        ke[h] = np.tile(np.stack([v1, v2, one, one]), (1, B))
        qe[h] = np.tile(np.stack([one, one, w1, w2]), (1, B))
    ins["qext"] = qe
    ins["kext"] = ke
    return ins


_NC_CACHE = {}


def _get_nc():
    if "nc" not in _NC_CACHE:
        _NC_CACHE["nc"] = build_nc()
    return _NC_CACHE["nc"]


def kernel(x, Wq, Wk, Wv, Wo):
    x = np.asarray(x, dtype=np.float32)
    Wq = np.asarray(Wq, dtype=np.float32)
    Wk = np.asarray(Wk, dtype=np.float32)
    Wv = np.asarray(Wv, dtype=np.float32)
    Wo = np.asarray(Wo, dtype=np.float32)
    assert x.shape == (B, T, D), x.shape

    nc = _get_nc()
    in_maps = [make_core_inputs(x, Wq, Wk, Wv, Wo, c) for c in range(N_CORES)]
    res = bass_utils.run_bass_kernel_spmd(nc, in_maps,
                                          core_ids=list(range(N_CORES)))
    acc = np.zeros((8, P, ECH, 512), np.float32)
    for c in range(N_CORES):
        acc += res.results[c]["yT"].astype(np.float32)
    # [TI, P, ECH, 512] -> y^T [D, BT]: yT[blk, p, e, c] = y^T[e*128+p, 512*blk+c]
    yt = acc.transpose(2, 1, 0, 3).reshape(D, B * T)
    return np.ascontiguousarray(yt.T).reshape(B, T, D)
